# revision 1
# baseline (speedup 1.0000x reference)
"""GCN (3-layer, edge-weighted, mean-pool, classifier) on 8 TRN2 NeuronCores.

Strategy (sharding_hint: shard nodes + incident edges across cores):
- Nodes are assigned to 8 cores round-robin by in-degree rank, so each
  core gets ~6250 nodes in 49 blocks of 128 with near-uniform in-degree
  per block.  Each target node owns K slots (its in-edges incl. the
  self-loop, padded to the block max K_b).
- norm = dis[src] * w_e * dis[tgt] is factored: dis[src] is folded into
  the gathered table (h~ = dis * h), w_e is applied per-slot on DVE,
  dis[tgt] is applied per-partition after aggregation.
- Per layer: every core gathers h~[src] rows from a replicated DRAM
  table (one indirect DMA per 128-edge chunk), multiplies by w, reduces
  slots on DVE, then per 128-node block: transpose (PE), augmented
  matmul with [W; b] (bias via ones row), relu*dis on ACT.  Layers
  exchange h~ via AllGather.  Pooling = one-hot graph matmul into an
  accumulating PSUM bank, AllReduce, then a tiny classifier matmul.
"""
import sys

for p in ("/opt/trn_rl_repo", "/root/.axon_site/_ro/trn_rl_repo"):
    if p not in sys.path:
        sys.path.insert(0, p)

import numpy as np

import concourse.bacc as bacc
import concourse.bass as bass
import concourse.mybir as mybir
import concourse.tile as tile
from concourse import bass_utils
from concourse.masks import make_identity

N_NODES = 50000
N_EDGES = 800000
F = 64
N_CLASSES = 10
N_GRAPHS = 64
NC = 8
NPC = 6272                # node slots per core (49 blocks of 128)
NB = NPC // 128           # 49
SLOTS = NC * NPC          # 50176

_cache = {}


def _host_prep(x, edge_index, batch, P_vec):
    """Slot layout + per-core input arrays (pure index manipulation)."""
    row = np.asarray(edge_index[0], np.int64)
    col = np.asarray(edge_index[1], np.int64)
    batch = np.asarray(batch, np.int64)
    P_vec = np.asarray(P_vec, np.float32)
    x = np.asarray(x, np.float32)

    deg = np.bincount(col, minlength=N_NODES)      # self loop handled on-chip
    order = np.argsort(-deg, kind="stable")        # nodes by degree desc
    r_of_node = np.empty(N_NODES, np.int64)
    r_of_node[order] = np.arange(N_NODES)
    core_of = r_of_node % NC
    pos_of = r_of_node // NC                       # < 6250
    slotrow_of = core_of * NPC + pos_of

    # graph edges only (self loops are the identity contribution, added
    # from the core's own SBUF h~ block); w = sigmoid(P)
    esrc = row
    etgt = col
    eP = P_vec

    # slot rank k of each edge within its target
    o = np.argsort(etgt, kind="stable")
    sk = etgt[o]
    grp_first = np.r_[True, sk[1:] != sk[:-1]]
    gstart = np.flatnonzero(grp_first)
    glen = np.diff(np.r_[gstart, len(sk)])
    kslot_sorted = np.arange(len(sk)) - np.repeat(gstart, glen)
    kslot = np.empty(len(sk), np.int64)
    kslot[o] = kslot_sorted

    # per-block chunk count (global across cores -> SPMD-uniform program)
    block_of_node = pos_of // 128
    Kb = np.zeros(NB, np.int64)
    np.maximum.at(Kb, block_of_node, deg)
    Kb = np.maximum(Kb, 1)
    cbase = np.r_[0, np.cumsum(Kb)][:-1]
    C = int(Kb.sum())

    tcore = core_of[etgt]
    tlane = pos_of[etgt] % 128
    ccol = cbase[block_of_node[etgt]] + kslot

    idx_arr = np.zeros((NC, 128, C), np.int32)
    P_arr = np.full((NC, 128, C), -1e4, np.float32)  # pad: sigmoid -> ~0
    idx_arr[tcore, tlane, ccol] = slotrow_of[esrc]
    P_arr[tcore, tlane, ccol] = eP

    gid_arr = np.full((NC, 128, NB), float(N_GRAPHS), np.float32)
    gid_arr[core_of, pos_of % 128, pos_of // 128] = batch.astype(np.float32)

    x_slots = np.zeros((NC, NPC, F), np.float32)
    x_slots[core_of, pos_of] = x

    return dict(
        Kb=[int(k) for k in Kb], cbase=[int(c) for c in cbase], C=C,
        idx=idx_arr, P=P_arr, gid=gid_arr, x_slots=x_slots,
    )


def _build(Kb, cbase, C):
    f32 = mybir.dt.float32
    nc = bacc.Bacc("TRN2", target_bir_lowering=False, debug=False, num_devices=NC)

    x_own = nc.dram_tensor("x_own", [NPC, F], f32, kind="ExternalInput")
    idx_in = nc.dram_tensor("idx", [128, C], mybir.dt.int32, kind="ExternalInput")
    p_in = nc.dram_tensor("pv", [128, C], f32, kind="ExternalInput")
    gid_in = nc.dram_tensor("gid", [128, NB], f32, kind="ExternalInput")
    iota_in = nc.dram_tensor("iota64", [128, F], f32, kind="ExternalInput")
    waug_in = [nc.dram_tensor(f"waug{l}", [F + 1, F], f32, kind="ExternalInput")
               for l in range(3)]
    wl_in = nc.dram_tensor("wlaug", [F + 1, N_CLASSES], f32, kind="ExternalInput")
    out_d = nc.dram_tensor("out", [N_GRAPHS, N_CLASSES], f32, kind="ExternalOutput")

    with tile.TileContext(nc) as tc:
        with tc.tile_pool(name="const", bufs=1) as cp, \
             tc.tile_pool(name="meta", bufs=1) as mp, \
             tc.tile_pool(name="work", bufs=3) as wp, \
             tc.tile_pool(name="msgs", bufs=4) as gp, \
             tc.tile_pool(name="psA", bufs=2, space="PSUM") as psA, \
             tc.tile_pool(name="psB", bufs=2, space="PSUM") as psB, \
             tc.tile_pool(name="psP", bufs=1, space="PSUM") as psP, \
             tc.tile_pool(name="dram", bufs=1, space="DRAM") as dp:

            ident = cp.tile([128, 128], f32)
            make_identity(nc, ident[:])
            iota_sb = cp.tile([128, F], f32)
            nc.sync.dma_start(out=iota_sb[:], in_=iota_in[:, :])
            waug_sb = []
            for l in range(3):
                t = cp.tile([F + 1, F], f32, tag=f"waug{l}")
                nc.sync.dma_start(out=t[:], in_=waug_in[l][:, :])
                waug_sb.append(t)
            wl_sb = cp.tile([F + 1, N_CLASSES], f32)
            nc.sync.dma_start(out=wl_sb[:], in_=wl_in[:, :])

            idx_sb = mp.tile([128, C], mybir.dt.int32)
            nc.sync.dma_start(out=idx_sb[:], in_=idx_in[:, :])
            w_sb = mp.tile([128, C], f32)
            gid_sb = mp.tile([128, NB], f32)
            nc.sync.dma_start(out=gid_sb[:], in_=gid_in[:, :])
            dis_sb = mp.tile([128, NB], f32)

            # ---- prepass: w = sigmoid(P); dis = 1/sqrt(deg_w + 1); x~ ----
            p_sb = wp.tile([128, C], f32, tag="ptmp")
            nc.sync.dma_start(out=p_sb[:], in_=p_in[:, :])
            nc.scalar.activation(out=w_sb[:], in_=p_sb[:],
                                 func=mybir.ActivationFunctionType.Sigmoid)
            deg_sb = wp.tile([128, NB], f32, tag="deg")
            for b in range(NB):
                nc.vector.tensor_reduce(
                    out=deg_sb[:, b:b + 1],
                    in_=w_sb[:, cbase[b]:cbase[b] + Kb[b]],
                    axis=mybir.AxisListType.X, op=mybir.AluOpType.add)
            # + 1.0 for the self loop (weight exactly 1), sqrt on ACT
            nc.scalar.activation(out=deg_sb[:], in_=deg_sb[:],
                                 func=mybir.ActivationFunctionType.Sqrt,
                                 bias=1.0)
            nc.vector.reciprocal(out=dis_sb[:], in_=deg_sb[:])

            hout = mp.tile([128, NB * F], f32)  # per-core h~ blocks
            xs = wp.tile([128, NB * F], f32, tag="xload")
            nc.sync.dma_start(
                out=xs[:].rearrange("p (b f) -> p b f", f=F),
                in_=x_own[:, :].rearrange("(b p) f -> p b f", p=128))
            nc.vector.tensor_tensor(
                out=hout[:].rearrange("p (b f) -> p b f", f=F),
                in0=xs[:].rearrange("p (b f) -> p b f", f=F),
                in1=dis_sb[:].to_broadcast([128, NB, F]),
                op=mybir.AluOpType.mult)

            agins = [dp.tile([NPC, F], f32, name=f"agin{l}", tag=f"agin{l}")
                     for l in range(3)]
            agouts = [dp.tile([SLOTS, F], f32, addr_space="Shared",
                              name=f"agout{l}", tag=f"ag{l}") for l in range(3)]
            pool_ps = psP.tile([N_GRAPHS, F + 1], f32)

            nc.sync.dma_start(
                out=agins[0][:].rearrange("(b p) f -> p b f", p=128),
                in_=hout[:].rearrange("p (b f) -> p b f", f=F))
            for l in range(3):
                # layer 1/2 bounce buffers were already filled per-block by
                # the previous layer's epilogue DMAs
                nc.gpsimd.collective_compute(
                    "AllGather", mybir.AluOpType.bypass,
                    ins=[agins[l][:]], outs=[agouts[l][:]],
                    replica_groups=[list(range(NC))])
                src = agouts[l]

                for b in range(NB):
                    K = Kb[b]
                    msg = gp.tile([128, K * F], f32, tag="msg")
                    for k in range(K):
                        c = cbase[b] + k
                        nc.gpsimd.indirect_dma_start(
                            out=msg[:, k * F:(k + 1) * F],
                            out_offset=None,
                            in_=src[:],
                            in_offset=bass.IndirectOffsetOnAxis(
                                ap=idx_sb[:, c:c + 1], axis=0))
                    nc.vector.tensor_tensor(
                        out=msg[:].rearrange("p (k f) -> p k f", f=F),
                        in0=msg[:].rearrange("p (k f) -> p k f", f=F),
                        in1=w_sb[:, cbase[b]:cbase[b] + K].to_broadcast([128, K, F]),
                        op=mybir.AluOpType.mult)
                    agg = wp.tile([128, F], f32, tag="agg")
                    nc.vector.tensor_reduce(
                        out=agg[:],
                        in_=msg[:].rearrange("p (k f) -> p f k", f=F),
                        axis=mybir.AxisListType.X, op=mybir.AluOpType.add)
                    # self-loop: w=1 contribution is the core's own h~ block
                    nc.vector.tensor_tensor(
                        out=agg[:], in0=agg[:],
                        in1=hout[:, b * F:(b + 1) * F],
                        op=mybir.AluOpType.add)
                    nc.vector.tensor_scalar_mul(agg[:], agg[:], dis_sb[:, b:b + 1])
                    tp = psA.tile([F, 128], f32, tag="tp")
                    nc.tensor.transpose(out=tp[:], in_=agg[:], identity=ident[:])
                    aug = wp.tile([F + 1, 128], f32, tag="aug")
                    nc.vector.memset(aug[F:F + 1, :], 1.0)
                    nc.vector.tensor_copy(out=aug[:F, :], in_=tp[:])
                    gps = psB.tile([128, F], f32, tag="g")
                    nc.tensor.matmul(out=gps[:], lhsT=aug[:], rhs=waug_sb[l][:],
                                     start=True, stop=True)
                    if l < 2:
                        nc.scalar.activation(
                            out=hout[:, b * F:(b + 1) * F], in_=gps[:],
                            func=mybir.ActivationFunctionType.Relu,
                            scale=dis_sb[:, b:b + 1])
                        # ship this block to the next layer's AG bounce now,
                        # off the layer-boundary critical path
                        nc.sync.dma_start(
                            out=agins[l + 1][:].rearrange(
                                "(bb p) f -> p bb f", p=128)[:, b:b + 1, :],
                            in_=hout[:, b * F:(b + 1) * F])
                    else:
                        h3 = wp.tile([128, F + 1], f32, tag="h3")
                        nc.vector.memset(h3[:, F:F + 1], 1.0)
                        nc.vector.tensor_copy(out=h3[:, :F], in_=gps[:])
                        gmat = wp.tile([128, N_GRAPHS], f32, tag="gmat")
                        nc.vector.tensor_tensor(
                            out=gmat[:],
                            in0=gid_sb[:, b:b + 1].to_broadcast([128, N_GRAPHS]),
                            in1=iota_sb[:],
                            op=mybir.AluOpType.is_equal)
                        nc.tensor.matmul(out=pool_ps[:], lhsT=gmat[:], rhs=h3[:],
                                         start=(b == 0), stop=(b == NB - 1))

            # ---- pooling epilogue ----
            poolin = wp.tile([N_GRAPHS, F + 1], f32, tag="poolin")
            nc.vector.tensor_copy(out=poolin[:], in_=pool_ps[:])
            arin = dp.tile([N_GRAPHS, F + 1], f32, tag="arin")
            arout = dp.tile([N_GRAPHS, F + 1], f32, addr_space="Shared", tag="arout")
            nc.sync.dma_start(out=arin[:], in_=poolin[:])
            nc.gpsimd.collective_compute(
                "AllReduce", mybir.AluOpType.add,
                ins=[arin[:]], outs=[arout[:]],
                replica_groups=[list(range(NC))])
            ar_sb = wp.tile([N_GRAPHS, F + 1], f32, tag="arsb")
            nc.sync.dma_start(out=ar_sb[:], in_=arout[:])
            cnt = wp.tile([N_GRAPHS, 1], f32, tag="cnt")
            nc.vector.tensor_scalar_max(cnt[:], ar_sb[:, F:F + 1], 1.0)
            rec = wp.tile([N_GRAPHS, 1], f32, tag="rec")
            nc.vector.reciprocal(out=rec[:], in_=cnt[:])
            pooled = wp.tile([N_GRAPHS, F], f32, tag="pooled")
            nc.vector.tensor_scalar_mul(pooled[:], ar_sb[:, :F], rec[:])
            tp2 = psA.tile([F, N_GRAPHS], f32, tag="tp")
            nc.tensor.transpose(out=tp2[:], in_=pooled[:],
                                identity=ident[:N_GRAPHS, :N_GRAPHS])
            aug2 = wp.tile([F + 1, N_GRAPHS], f32, tag="aug2")
            nc.vector.memset(aug2[F:F + 1, :], 1.0)
            nc.vector.tensor_copy(out=aug2[:F, :], in_=tp2[:])
            ops = psB.tile([N_GRAPHS, N_CLASSES], f32, tag="g")
            nc.tensor.matmul(out=ops[:], lhsT=aug2[:], rhs=wl_sb[:],
                             start=True, stop=True)
            out_sb = wp.tile([N_GRAPHS, N_CLASSES], f32, tag="outsb")
            nc.vector.tensor_copy(out=out_sb[:], in_=ops[:])
            nc.sync.dma_start(out=out_d[:, :], in_=out_sb[:])

    nc.compile()
    return nc


def _run(inputs, trace=False):
    x = inputs["x"]
    prep = _host_prep(x, inputs["edge_index"], inputs["batch"], inputs["P_vec"])
    key = ("nc", prep["C"], tuple(prep["Kb"]))
    if key not in _cache:
        _cache.clear()
        _cache[key] = _build(prep["Kb"], prep["cbase"], prep["C"])
    nc = _cache[key]

    waugs = []
    for (W, b) in [(inputs["W1"], inputs["b1"]), (inputs["W2"], inputs["b2"]),
                   (inputs["W3"], inputs["b3"])]:
        waugs.append(np.concatenate(
            [np.asarray(W, np.float32), np.asarray(b, np.float32)[None, :]], axis=0))
    wlaug = np.concatenate(
        [np.asarray(inputs["Wl"], np.float32),
         np.asarray(inputs["bl"], np.float32)[None, :]], axis=0)
    iota64 = np.tile(np.arange(F, dtype=np.float32)[None, :], (128, 1))

    in_maps = []
    for c in range(NC):
        in_maps.append({
            "x_own": prep["x_slots"][c],
            "idx": prep["idx"][c], "pv": prep["P"][c],
            "gid": prep["gid"][c], "iota64": iota64,
            "waug0": waugs[0], "waug1": waugs[1], "waug2": waugs[2],
            "wlaug": wlaug,
        })

    res = bass_utils.run_bass_kernel_spmd(
        nc, in_maps, core_ids=list(range(NC)), trace=trace)
    return res.results[0]["out"].astype(np.float32), res


def kernel(**inputs) -> np.ndarray:
    out, _ = _run(inputs, trace=False)
    return out



# revision 19
# speedup vs baseline: 1.1951x; 1.1951x over previous
"""GCN (3-layer, edge-weighted, mean-pool, classifier) on 8 TRN2 NeuronCores.

Strategy (sharding_hint: shard nodes + incident edges across cores):
- Nodes are assigned to 8 cores round-robin by in-degree rank, so each
  core gets ~6250 nodes in 49 blocks of 128 with near-uniform in-degree
  per block.  Each target node owns K slots (its in-edges + self loop,
  padded to the block max K_b).
- norm = dis[src] * w_e * dis[tgt] is factored: dis[src] is folded into
  the gathered table (h~ = dis * h), w_e is applied per-slot on DVE,
  dis[tgt] is applied per-partition after aggregation.  The self loop
  is an ordinary slot whose weight sigmoid(30) == 1.0 exactly.
- The h~ table is bf16 and gathered in PAIRS of rows (256B elements,
  the SWDGE dma_gather granularity) with int16 indices slotrow//2; the
  unwanted half of each pair is masked by a zero weight (host pads the
  P array with -1e4, sigmoid -> 0).  One dma_gather per ~128-slot chunk
  replaces per-slot indirect DMAs (~1us fixed SWDGE cost each).
- Per layer: chunked gathers, DVE multiply+reduce, then per 128-node
  block: transpose (PE), augmented matmul with [W; b], relu*dis on ACT.
  Layers exchange h~ via AllGather.  Pooling = one-hot graph matmul
  into an accumulating PSUM bank, AllReduce, tiny classifier matmul.
"""
import sys

for p in ("/opt/trn_rl_repo", "/root/.axon_site/_ro/trn_rl_repo"):
    if p not in sys.path:
        sys.path.insert(0, p)

import numpy as np

import concourse.bacc as bacc
import concourse.bass as bass
import concourse.mybir as mybir
import concourse.tile as tile
from concourse import bass_utils
from concourse.library_config import mlp
from concourse.masks import make_identity

N_NODES = 50000
N_EDGES = 800000
F = 64
N_CLASSES = 10
N_GRAPHS = 64
NC = 8
NPC = 6272                # node slots per core (49 blocks of 128)
NB = NPC // 128           # 49
SLOTS = NC * NPC          # 50176
KCAP = 64                 # slot columns per span (DVE granularity)
GSUB = 8                  # columns per dma_gather (1024 idx ucode limit)
SELF_P = 30.0             # sigmoid(30) == 1.0 exactly in f32

_cache = {}


def _host_prep(x, edge_index, batch, P_vec):
    """Slot layout + per-core input arrays (pure index manipulation)."""
    row = np.asarray(edge_index[0], np.int64)
    col = np.asarray(edge_index[1], np.int64)
    batch = np.asarray(batch, np.int64)
    P_vec = np.asarray(P_vec, np.float32)
    x = np.asarray(x, np.float32)

    deg = np.bincount(col, minlength=N_NODES) + 1   # + self loop slot
    order = np.argsort(-deg, kind="stable")         # nodes by degree desc
    r_of_node = np.empty(N_NODES, np.int64)
    r_of_node[order] = np.arange(N_NODES)
    core_of = r_of_node % NC
    pos_of = r_of_node // NC                        # < 6250
    slotrow_of = core_of * NPC + pos_of

    # graph edges + one self loop per node (weight sigmoid(30) == 1.0)
    esrc = np.concatenate([row, np.arange(N_NODES)])
    etgt = np.concatenate([col, np.arange(N_NODES)])
    eP = np.concatenate([P_vec, np.full(N_NODES, SELF_P, np.float32)])

    # slot rank k of each edge within its target
    o = np.argsort(etgt, kind="stable")
    sk = etgt[o]
    grp_first = np.r_[True, sk[1:] != sk[:-1]]
    gstart = np.flatnonzero(grp_first)
    glen = np.diff(np.r_[gstart, len(sk)])
    kslot_sorted = np.arange(len(sk)) - np.repeat(gstart, glen)
    kslot = np.empty(len(sk), np.int64)
    kslot[o] = kslot_sorted

    # per-block chunk count (global across cores -> SPMD-uniform program)
    block_of_node = pos_of // 128
    Kb = np.zeros(NB, np.int64)
    np.maximum.at(Kb, block_of_node, deg)
    Kb = np.maximum(Kb, 1)
    cbase = np.r_[0, np.cumsum(Kb)][:-1]
    C = int(Kb.sum())

    tcore = core_of[etgt]
    tlane = pos_of[etgt] % 128
    ccol = cbase[block_of_node[etgt]] + kslot

    src_slot = slotrow_of[esrc]
    idx_arr = np.zeros((NC, 128, C), np.int32)      # pair index slotrow//2
    P2_arr = np.full((NC, 128, 2 * C), -1e4, np.float32)
    idx_arr[tcore, tlane, ccol] = src_slot // 2
    P2_arr[tcore, tlane, 2 * ccol + (src_slot % 2)] = eP
    # pad lanes (no real node) get one unit-weight slot pointing at pair 0
    # so deg == 1 there instead of 0 (avoids inf/NaN downstream)
    for pos in range(N_NODES // NC, NPC):
        P2_arr[:, pos % 128, 2 * cbase[pos // 128]] = SELF_P

    # dma_gather int16 index layout: flat i = c*128 + p stored at
    # partition i%16, column i//16, replicated to all 8 16-row groups
    i16 = np.zeros((NC, 16, C * 8), np.int16)
    flat = np.transpose(idx_arr, (0, 2, 1)).reshape(NC, C * 128)  # (c,p)
    u16 = flat.astype(np.uint16).view(np.int16)
    ii = np.arange(C * 128)
    i16[:, ii % 16, ii // 16] = u16
    idx16 = np.tile(i16, (1, 8, 1))                  # [NC, 128, C*8]

    gid_arr = np.full((NC, 128, NB), float(N_GRAPHS), np.float32)
    gid_arr[core_of, pos_of % 128, pos_of // 128] = batch.astype(np.float32)

    x_slots = np.zeros((NC, NPC, F), np.float32)
    x_slots[core_of, pos_of] = x

    return dict(
        Kb=[int(k) for k in Kb], cbase=[int(c) for c in cbase], C=C,
        idx16=idx16, P2=P2_arr, gid=gid_arr, x_slots=x_slots,
    )


def _spans(Kb, cbase, C):
    """Column spans of <= KCAP plus per-span block pieces.

    Returns [(c0, c1, [(b, s, e, first, last), ...]), ...]: block b's
    slot columns [s, e) fall in this span; first/last flag whether the
    piece begins/ends b's range (for partial-sum stitching).
    """
    out = []
    c = 0
    while c < C:
        c0, c1 = c, min(c + KCAP, C)
        plist = []
        for b in range(NB):
            s = max(cbase[b], c0)
            e = min(cbase[b] + Kb[b], c1)
            if s < e:
                plist.append((b, s, e, s == cbase[b],
                              e == cbase[b] + Kb[b]))
        out.append((c0, c1, plist))
        c = c1
    return out


def _build(Kb, cbase, C):
    f32 = mybir.dt.float32
    bf16 = mybir.dt.bfloat16
    i16 = mybir.dt.int16
    nc = bacc.Bacc("TRN2", target_bir_lowering=False, debug=False,
                   num_devices=NC)

    x_own = nc.dram_tensor("x_own", [NPC, F], f32, kind="ExternalInput")
    idx_in = nc.dram_tensor("idx16", [128, C * 8], i16, kind="ExternalInput")
    p_in = nc.dram_tensor("pv", [128, 2 * C], f32, kind="ExternalInput")
    gid_in = nc.dram_tensor("gid", [128, NB], f32, kind="ExternalInput")
    iota_in = nc.dram_tensor("iota64", [128, F], f32, kind="ExternalInput")
    waug_in = [nc.dram_tensor(f"waug{l}", [F + 1, F], f32, kind="ExternalInput")
               for l in range(3)]
    wl_in = nc.dram_tensor("wlaug", [F + 1, N_CLASSES], f32, kind="ExternalInput")
    out_d = nc.dram_tensor("out", [N_GRAPHS, N_CLASSES], f32, kind="ExternalOutput")

    spans = _spans(Kb, cbase, C)

    with tile.TileContext(nc) as tc:
        with tc.tile_pool(name="const", bufs=1) as cp, \
             tc.tile_pool(name="meta", bufs=1) as mp, \
             tc.tile_pool(name="work", bufs=3) as wp, \
             tc.tile_pool(name="msgs", bufs=4) as gp, \
             tc.tile_pool(name="psA", bufs=3, space="PSUM") as psA, \
             tc.tile_pool(name="psB", bufs=3, space="PSUM") as psB, \
             tc.tile_pool(name="psP", bufs=1, space="PSUM") as psP, \
             tc.tile_pool(name="dram", bufs=1, space="DRAM") as dp:

            nc.gpsimd.load_library(mlp)
            ident = cp.tile([128, 128], f32)
            make_identity(nc, ident[:])
            iota_sb = cp.tile([128, F], f32)
            nc.sync.dma_start(out=iota_sb[:], in_=iota_in[:, :])
            waug_sb = []
            for l in range(3):
                t = cp.tile([F + 1, F], f32, tag=f"waug{l}")
                nc.sync.dma_start(out=t[:], in_=waug_in[l][:, :])
                waug_sb.append(t)
            wl_sb = cp.tile([F + 1, N_CLASSES], f32)
            nc.sync.dma_start(out=wl_sb[:], in_=wl_in[:, :])

            idx_sb = mp.tile([128, C * 8], i16)
            nc.sync.dma_start(out=idx_sb[:], in_=idx_in[:, :])
            w2_sb = mp.tile([128, 2 * C], f32)
            w2b_sb = mp.tile([128, 2 * C], bf16)
            gid_sb = mp.tile([128, NB], f32)
            nc.sync.dma_start(out=gid_sb[:], in_=gid_in[:, :])
            dis_sb = mp.tile([128, NB], f32)

            # ---- prepass: w = sigmoid(P); dis = 1/sqrt(deg_w); x~ ----
            p_sb = gp.tile([128, 2 * C], f32, tag="msg")
            nc.sync.dma_start(out=p_sb[:], in_=p_in[:, :])
            nc.scalar.activation(out=w2_sb[:], in_=p_sb[:],
                                 func=mybir.ActivationFunctionType.Sigmoid)
            nc.vector.tensor_copy(out=w2b_sb[:], in_=w2_sb[:])
            deg_sb = wp.tile([128, NB], f32, tag="deg")
            for b in range(NB):
                nc.vector.tensor_reduce(
                    out=deg_sb[:, b:b + 1],
                    in_=w2_sb[:, 2 * cbase[b]:2 * (cbase[b] + Kb[b])],
                    axis=mybir.AxisListType.X, op=mybir.AluOpType.add)
            # self-loop weight is included as a slot, so deg is complete
            # (pad lanes get a unit slot host-side, so deg >= 1 everywhere)
            nc.scalar.activation(out=deg_sb[:], in_=deg_sb[:],
                                 func=mybir.ActivationFunctionType.Sqrt)
            nc.vector.reciprocal(out=dis_sb[:], in_=deg_sb[:])

            xs = gp.tile([128, NB * F], f32, tag="msg")
            nc.sync.dma_start(
                out=xs[:].rearrange("p (b f) -> p b f", f=F),
                in_=x_own[:, :].rearrange("(b p) f -> p b f", p=128))
            xb = gp.tile([128, NB * F], bf16, tag="msg")
            nc.vector.tensor_tensor(
                out=xb[:].rearrange("p (b f) -> p b f", f=F),
                in0=xs[:].rearrange("p (b f) -> p b f", f=F),
                in1=dis_sb[:].to_broadcast([128, NB, F]),
                op=mybir.AluOpType.mult)

            agins = [dp.tile([NPC, F], bf16, name=f"agin{l}", tag=f"agin{l}")
                     for l in range(3)]
            agouts = [dp.tile([SLOTS, F], bf16, addr_space="Shared",
                              name=f"agout{l}", tag=f"ag{l}") for l in range(3)]
            pool_ps = psP.tile([N_GRAPHS, F + 1], f32)

            nc.sync.dma_start(
                out=agins[0][:].rearrange("(b p) f -> p b f", p=128),
                in_=xb[:].rearrange("p (b f) -> p b f", f=F))
            for l in range(3):
                # layer 1/2 bounce buffers were already filled per-block by
                # the previous layer's epilogue DMAs
                nc.gpsimd.collective_compute(
                    "AllGather", mybir.AluOpType.bypass,
                    ins=[agins[l][:]], outs=[agouts[l][:]],
                    replica_groups=[list(range(NC))])
                # gather source viewed as pair rows [SLOTS/2, 2F]
                src = agouts[l][:].rearrange("(j t) f -> j (t f)", t=2)

                open_agg = None
                for (c0, c1, plist) in spans:
                    Kc = c1 - c0
                    msg = gp.tile([128, Kc * 2 * F], bf16, tag="msg")
                    for g0 in range(c0, c1, GSUB):
                        g1 = min(g0 + GSUB, c1)
                        Kg = g1 - g0
                        nc.gpsimd.dma_gather(
                            msg[:, (g0 - c0) * 2 * F:(g1 - c0) * 2 * F]
                            .rearrange("p (k f) -> p k f", f=2 * F),
                            src,
                            idx_sb[:, g0 * 8:g1 * 8],
                            Kg * 128, Kg * 128, 2 * F)
                    nc.vector.tensor_tensor(
                        out=msg[:].rearrange("p (q f) -> p q f", f=F),
                        in0=msg[:].rearrange("p (q f) -> p q f", f=F),
                        in1=w2b_sb[:, 2 * c0:2 * c1].to_broadcast(
                            [128, 2 * Kc, F]),
                        op=mybir.AluOpType.mult)
                    for (b, s, e, first, last) in plist:
                        o = (s - c0) * 2 * F
                        n = (e - s) * 2 * F
                        piece = msg[:, o:o + n].rearrange(
                            "p (q f) -> p f q", f=F)
                        if first:
                            agg = wp.tile([128, F], f32, tag="agg")
                            nc.vector.tensor_reduce(
                                out=agg[:], in_=piece,
                                axis=mybir.AxisListType.X,
                                op=mybir.AluOpType.add)
                        else:
                            agg = open_agg
                            tmp = wp.tile([128, F], f32, tag="aggt")
                            nc.vector.tensor_reduce(
                                out=tmp[:], in_=piece,
                                axis=mybir.AxisListType.X,
                                op=mybir.AluOpType.add)
                            nc.vector.tensor_tensor(
                                out=agg[:], in0=agg[:], in1=tmp[:],
                                op=mybir.AluOpType.add)
                        if not last:
                            open_agg = agg
                            continue
                        open_agg = None
                        nc.vector.tensor_scalar_mul(
                            agg[:], agg[:], dis_sb[:, b:b + 1])
                        tp = psA.tile([F, 128], f32, tag="tp")
                        nc.tensor.transpose(out=tp[:], in_=agg[:],
                                            identity=ident[:])
                        aug = wp.tile([F + 1, 128], f32, tag="aug")
                        nc.vector.memset(aug[F:F + 1, :], 1.0)
                        nc.vector.tensor_copy(out=aug[:F, :], in_=tp[:])
                        gps = psB.tile([128, F], f32, tag="g")
                        nc.tensor.matmul(out=gps[:], lhsT=aug[:],
                                         rhs=waug_sb[l][:],
                                         start=True, stop=True)
                        if l < 2:
                            hb = wp.tile([128, F], bf16, tag="hb")
                            nc.scalar.activation(
                                out=hb[:], in_=gps[:],
                                func=mybir.ActivationFunctionType.Relu,
                                scale=dis_sb[:, b:b + 1])
                            # ship this block to the next layer's AG bounce
                            # now, off the layer-boundary critical path
                            nc.sync.dma_start(
                                out=agins[l + 1][:].rearrange(
                                    "(bb p) f -> p bb f", p=128)[:, b:b + 1, :],
                                in_=hb[:])
                        else:
                            h3 = wp.tile([128, F + 1], f32, tag="h3")
                            nc.vector.memset(h3[:, F:F + 1], 1.0)
                            nc.vector.tensor_copy(out=h3[:, :F], in_=gps[:])
                            gmat = wp.tile([128, N_GRAPHS], f32, tag="gmat")
                            nc.vector.tensor_tensor(
                                out=gmat[:],
                                in0=gid_sb[:, b:b + 1].to_broadcast(
                                    [128, N_GRAPHS]),
                                in1=iota_sb[:],
                                op=mybir.AluOpType.is_equal)
                            nc.tensor.matmul(out=pool_ps[:], lhsT=gmat[:],
                                             rhs=h3[:],
                                             start=(b == 0),
                                             stop=(b == NB - 1))

            # ---- pooling epilogue ----
            poolin = wp.tile([N_GRAPHS, F + 1], f32, tag="poolin")
            nc.vector.tensor_copy(out=poolin[:], in_=pool_ps[:])
            arin = dp.tile([N_GRAPHS, F + 1], f32, tag="arin")
            arout = dp.tile([N_GRAPHS, F + 1], f32, addr_space="Shared", tag="arout")
            nc.sync.dma_start(out=arin[:], in_=poolin[:])
            nc.gpsimd.collective_compute(
                "AllReduce", mybir.AluOpType.add,
                ins=[arin[:]], outs=[arout[:]],
                replica_groups=[list(range(NC))])
            ar_sb = wp.tile([N_GRAPHS, F + 1], f32, tag="arsb")
            nc.sync.dma_start(out=ar_sb[:], in_=arout[:])
            cnt = wp.tile([N_GRAPHS, 1], f32, tag="cnt")
            nc.vector.tensor_scalar_max(cnt[:], ar_sb[:, F:F + 1], 1.0)
            rec = wp.tile([N_GRAPHS, 1], f32, tag="rec")
            nc.vector.reciprocal(out=rec[:], in_=cnt[:])
            pooled = wp.tile([N_GRAPHS, F], f32, tag="pooled")
            nc.vector.tensor_scalar_mul(pooled[:], ar_sb[:, :F], rec[:])
            tp2 = psA.tile([F, N_GRAPHS], f32, tag="tp")
            nc.tensor.transpose(out=tp2[:], in_=pooled[:],
                                identity=ident[:N_GRAPHS, :N_GRAPHS])
            aug2 = wp.tile([F + 1, N_GRAPHS], f32, tag="aug2")
            nc.vector.memset(aug2[F:F + 1, :], 1.0)
            nc.vector.tensor_copy(out=aug2[:F, :], in_=tp2[:])
            ops = psB.tile([N_GRAPHS, N_CLASSES], f32, tag="g")
            nc.tensor.matmul(out=ops[:], lhsT=aug2[:], rhs=wl_sb[:],
                             start=True, stop=True)
            out_sb = wp.tile([N_GRAPHS, N_CLASSES], f32, tag="outsb")
            nc.vector.tensor_copy(out=out_sb[:], in_=ops[:])
            nc.sync.dma_start(out=out_d[:, :], in_=out_sb[:])

    nc.compile()
    return nc


def _run(inputs, trace=False):
    x = inputs["x"]
    prep = _host_prep(x, inputs["edge_index"], inputs["batch"], inputs["P_vec"])
    key = ("nc", prep["C"], tuple(prep["Kb"]))
    if key not in _cache:
        _cache.clear()
        _cache[key] = _build(prep["Kb"], prep["cbase"], prep["C"])
    nc = _cache[key]

    waugs = []
    for (W, b) in [(inputs["W1"], inputs["b1"]), (inputs["W2"], inputs["b2"]),
                   (inputs["W3"], inputs["b3"])]:
        waugs.append(np.concatenate(
            [np.asarray(W, np.float32), np.asarray(b, np.float32)[None, :]], axis=0))
    wlaug = np.concatenate(
        [np.asarray(inputs["Wl"], np.float32),
         np.asarray(inputs["bl"], np.float32)[None, :]], axis=0)
    iota64 = np.tile(np.arange(F, dtype=np.float32)[None, :], (128, 1))

    in_maps = []
    for c in range(NC):
        in_maps.append({
            "x_own": prep["x_slots"][c],
            "idx16": prep["idx16"][c], "pv": prep["P2"][c],
            "gid": prep["gid"][c], "iota64": iota64,
            "waug0": waugs[0], "waug1": waugs[1], "waug2": waugs[2],
            "wlaug": wlaug,
        })

    res = bass_utils.run_bass_kernel_spmd(
        nc, in_maps, core_ids=list(range(NC)), trace=trace)
    return res.results[0]["out"].astype(np.float32), res


def kernel(**inputs) -> np.ndarray:
    out, _ = _run(inputs, trace=False)
    return out


# revision 29
# speedup vs baseline: 2.5386x; 2.1242x over previous
"""GCN (3-layer, edge-weighted, mean-pool, classifier) on 8 TRN2 NeuronCores.

Strategy (sharding_hint: shard nodes + incident edges across cores):
- Nodes are assigned to 8 cores round-robin by in-degree rank, so each
  core gets ~6250 nodes in 49 blocks of 128 with near-uniform in-degree
  per block.  Each target node owns K slots (its in-edges + self loop,
  padded to the block max K_b).
- norm = dis[src] * w_e * dis[tgt] is factored: dis[src] is folded into
  the gathered table (h~ = dis * h), w_e is applied per-slot on DVE,
  dis[tgt] is applied per-partition after aggregation.  The self loop
  is an ordinary slot whose weight sigmoid(30) == 1.0 exactly.
- The h~ table is bf16 and gathered in PAIRS of rows (256B elements,
  the SWDGE dma_gather granularity) with int16 indices slotrow//2; the
  unwanted half of each pair is masked by a zero weight (host pads the
  P array with -1e4, sigmoid -> 0).  One dma_gather per ~128-slot chunk
  replaces per-slot indirect DMAs (~1us fixed SWDGE cost each).
- Per layer: chunked gathers, DVE multiply+reduce, then per 128-node
  block: transpose (PE), augmented matmul with [W; b], relu*dis on ACT.
  Layers exchange h~ via AllGather.  Pooling = one-hot graph matmul
  into an accumulating PSUM bank, AllReduce, tiny classifier matmul.
"""
import sys

for p in ("/opt/trn_rl_repo", "/root/.axon_site/_ro/trn_rl_repo"):
    if p not in sys.path:
        sys.path.insert(0, p)

import numpy as np

import concourse.bacc as bacc
import concourse.bass as bass
import concourse.mybir as mybir
import concourse.tile as tile
from concourse import bass_utils
from concourse.library_config import mlp
from concourse.masks import make_identity

N_NODES = 50000
N_EDGES = 800000
F = 64
N_CLASSES = 10
N_GRAPHS = 64
NC = 8
NPC = 6272                # node slots per core (49 blocks of 128)
NB = NPC // 128           # 49
SLOTS = NC * NPC          # 50176
KCAP = 64                 # slot columns per span (DVE granularity)
GSUB = 8                  # columns per dma_gather (1024 idx ucode limit)
SELF_P = 30.0             # sigmoid(30) == 1.0 exactly in f32

_cache = {}


def _host_prep(x, edge_index, batch, P_vec):
    """Slot layout + per-core input arrays (pure index manipulation)."""
    row = np.asarray(edge_index[0], np.int64)
    col = np.asarray(edge_index[1], np.int64)
    batch = np.asarray(batch, np.int64)
    P_vec = np.asarray(P_vec, np.float32)
    x = np.asarray(x, np.float32)

    deg = np.bincount(col, minlength=N_NODES)       # self loop added on-chip
    order = np.argsort(-deg, kind="stable")         # nodes by degree desc
    r_of_node = np.empty(N_NODES, np.int64)
    r_of_node[order] = np.arange(N_NODES)
    core_of = r_of_node % NC
    pos_of = r_of_node // NC                        # < 6250
    slotrow_of = core_of * NPC + pos_of

    # graph edges only; the self loop (weight exactly 1.0) is added on-chip
    # from the core's own h~ block, so it costs no gather descriptors
    esrc = row
    etgt = col
    eP = P_vec

    # slot rank k of each edge within its target
    o = np.argsort(etgt, kind="stable")
    sk = etgt[o]
    grp_first = np.r_[True, sk[1:] != sk[:-1]]
    gstart = np.flatnonzero(grp_first)
    glen = np.diff(np.r_[gstart, len(sk)])
    kslot_sorted = np.arange(len(sk)) - np.repeat(gstart, glen)
    kslot = np.empty(len(sk), np.int64)
    kslot[o] = kslot_sorted

    # per-block chunk count (global across cores -> SPMD-uniform program)
    block_of_node = pos_of // 128
    Kb = np.zeros(NB, np.int64)
    np.maximum.at(Kb, block_of_node, deg)
    Kb = np.maximum(Kb, 1)
    cbase = np.r_[0, np.cumsum(Kb)][:-1]
    C = int(Kb.sum())

    tcore = core_of[etgt]
    tlane = pos_of[etgt] % 128
    ccol = cbase[block_of_node[etgt]] + kslot

    src_slot = slotrow_of[esrc]
    idx_arr = np.zeros((NC, 128, C), np.int32)      # pair index slotrow//2
    P2_arr = np.full((NC, 128, 2 * C), -1e4, np.float32)
    idx_arr[tcore, tlane, ccol] = src_slot // 2
    P2_arr[tcore, tlane, 2 * ccol + (src_slot % 2)] = eP

    # dma_gather int16 index layout: flat i = c*128 + p stored at
    # partition i%16, column i//16, replicated to all 8 16-row groups
    i16 = np.zeros((NC, 16, C * 8), np.int16)
    flat = np.transpose(idx_arr, (0, 2, 1)).reshape(NC, C * 128)  # (c,p)
    u16 = flat.astype(np.uint16).view(np.int16)
    ii = np.arange(C * 128)
    i16[:, ii % 16, ii // 16] = u16
    idx16 = np.tile(i16, (1, 8, 1))                  # [NC, 128, C*8]

    gid_arr = np.full((NC, 128, NB), float(N_GRAPHS), np.float32)
    gid_arr[core_of, pos_of % 128, pos_of // 128] = batch.astype(np.float32)

    x_slots = np.zeros((NC, NPC, F), np.float32)
    x_slots[core_of, pos_of] = x

    return dict(
        Kb=[int(k) for k in Kb], cbase=[int(c) for c in cbase], C=C,
        idx16=idx16, P2=P2_arr, gid=gid_arr, x_slots=x_slots,
    )


def _spans(Kb, cbase, C):
    """Column spans of <= KCAP plus per-span block pieces.

    Returns [(c0, c1, [(b, s, e, first, last), ...]), ...]: block b's
    slot columns [s, e) fall in this span; first/last flag whether the
    piece begins/ends b's range (for partial-sum stitching).
    """
    out = []
    c = 0
    while c < C:
        c0, c1 = c, min(c + KCAP, C)
        plist = []
        for b in range(NB):
            s = max(cbase[b], c0)
            e = min(cbase[b] + Kb[b], c1)
            if s < e:
                plist.append((b, s, e, s == cbase[b],
                              e == cbase[b] + Kb[b]))
        out.append((c0, c1, plist))
        c = c1
    return out


def _build(Kb, cbase, C):
    f32 = mybir.dt.float32
    bf16 = mybir.dt.bfloat16
    i16 = mybir.dt.int16
    nc = bacc.Bacc("TRN2", target_bir_lowering=False, debug=False,
                   num_devices=NC, num_swdge_queues=4)

    x_own = nc.dram_tensor("x_own", [NPC, F], f32, kind="ExternalInput")
    idx_in = nc.dram_tensor("idx16", [128, C * 8], i16, kind="ExternalInput")
    p_in = nc.dram_tensor("pv", [128, 2 * C], f32, kind="ExternalInput")
    gid_in = nc.dram_tensor("gid", [128, NB], f32, kind="ExternalInput")
    iota_in = nc.dram_tensor("iota64", [128, F], f32, kind="ExternalInput")
    waug_in = [nc.dram_tensor(f"waug{l}", [F + 1, F], f32, kind="ExternalInput")
               for l in range(3)]
    wl_in = nc.dram_tensor("wlaug", [F + 1, N_CLASSES], f32, kind="ExternalInput")
    out_d = nc.dram_tensor("out", [N_GRAPHS, N_CLASSES], f32, kind="ExternalOutput")

    spans = _spans(Kb, cbase, C)

    with tile.TileContext(nc) as tc:
        with tc.tile_pool(name="const", bufs=1) as cp, \
             tc.tile_pool(name="meta", bufs=1) as mp, \
             tc.tile_pool(name="work", bufs=3) as wp, \
             tc.tile_pool(name="msgs", bufs=4) as gp, \
             tc.tile_pool(name="psA", bufs=3, space="PSUM") as psA, \
             tc.tile_pool(name="psB", bufs=3, space="PSUM") as psB, \
             tc.tile_pool(name="psP", bufs=1, space="PSUM") as psP, \
             tc.tile_pool(name="dram", bufs=1, space="DRAM") as dp:

            nc.gpsimd.load_library(mlp)
            ident = cp.tile([128, 128], f32)
            make_identity(nc, ident[:])
            iota_sb = cp.tile([128, F], f32)
            nc.sync.dma_start(out=iota_sb[:], in_=iota_in[:, :])
            waug_sb = []
            for l in range(3):
                t = cp.tile([F + 1, F], f32, tag=f"waug{l}")
                nc.sync.dma_start(out=t[:], in_=waug_in[l][:, :])
                waug_sb.append(t)
            wl_sb = cp.tile([F + 1, N_CLASSES], f32)
            nc.sync.dma_start(out=wl_sb[:], in_=wl_in[:, :])

            idx_sb = mp.tile([128, C * 8], i16)
            nc.sync.dma_start(out=idx_sb[:], in_=idx_in[:, :])
            w2_sb = mp.tile([128, 2 * C], f32)
            w2b_sb = mp.tile([128, 2 * C], bf16)
            gid_sb = mp.tile([128, NB], f32)
            nc.sync.dma_start(out=gid_sb[:], in_=gid_in[:, :])
            dis_sb = mp.tile([128, NB], f32)

            # ---- prepass: w = sigmoid(P); dis = 1/sqrt(deg_w); x~ ----
            p_sb = gp.tile([128, 2 * C], f32, tag="msg")
            nc.sync.dma_start(out=p_sb[:], in_=p_in[:, :])
            nc.scalar.activation(out=w2_sb[:], in_=p_sb[:],
                                 func=mybir.ActivationFunctionType.Sigmoid)
            nc.vector.tensor_copy(out=w2b_sb[:], in_=w2_sb[:])
            deg_sb = wp.tile([128, NB], f32, tag="deg")
            for b in range(NB):
                nc.vector.tensor_reduce(
                    out=deg_sb[:, b:b + 1],
                    in_=w2_sb[:, 2 * cbase[b]:2 * (cbase[b] + Kb[b])],
                    axis=mybir.AxisListType.X, op=mybir.AluOpType.add)
            # + 1.0 for the self loop (weight exactly 1); also keeps pad
            # lanes (deg 0) finite
            nc.scalar.activation(out=deg_sb[:], in_=deg_sb[:],
                                 func=mybir.ActivationFunctionType.Sqrt,
                                 bias=1.0)
            nc.vector.reciprocal(out=dis_sb[:], in_=deg_sb[:])

            # persistent own h~ blocks (bf16): self-loop source + AG payload
            hslab = mp.tile([128, NB * F], bf16)
            xs = gp.tile([128, NB * F], f32, tag="msg")
            nc.sync.dma_start(
                out=xs[:].rearrange("p (b f) -> p b f", f=F),
                in_=x_own[:, :].rearrange("(b p) f -> p b f", p=128))
            nc.vector.tensor_tensor(
                out=hslab[:].rearrange("p (b f) -> p b f", f=F),
                in0=xs[:].rearrange("p (b f) -> p b f", f=F),
                in1=dis_sb[:].to_broadcast([128, NB, F]),
                op=mybir.AluOpType.mult)

            agins = [dp.tile([NPC, F], bf16, name=f"agin{l}", tag=f"agin{l}")
                     for l in range(3)]
            agouts = [dp.tile([SLOTS, F], bf16, addr_space="Shared",
                              name=f"agout{l}", tag=f"ag{l}") for l in range(3)]
            pool_ps = psP.tile([N_GRAPHS, F + 1], f32)

            nc.sync.dma_start(
                out=agins[0][:].rearrange("(b p) f -> p b f", p=128),
                in_=hslab[:].rearrange("p (b f) -> p b f", f=F))
            qi = 0
            for l in range(3):
                # layer 1/2 bounce buffers were already filled per-block by
                # the previous layer's epilogue DMAs
                nc.gpsimd.collective_compute(
                    "AllGather", mybir.AluOpType.bypass,
                    ins=[agins[l][:]], outs=[agouts[l][:]],
                    replica_groups=[list(range(NC))])
                # gather source viewed as pair rows [SLOTS/2, 2F]
                src = agouts[l][:].rearrange("(j t) f -> j (t f)", t=2)

                open_agg = None
                for (c0, c1, plist) in spans:
                    Kc = c1 - c0
                    msg = gp.tile([128, Kc * 2 * F], bf16, tag="msg")
                    for g0 in range(c0, c1, GSUB):
                        g1 = min(g0 + GSUB, c1)
                        Kg = g1 - g0
                        nc.gpsimd.dma_gather(
                            msg[:, (g0 - c0) * 2 * F:(g1 - c0) * 2 * F]
                            .rearrange("p (k f) -> p k f", f=2 * F),
                            src,
                            idx_sb[:, g0 * 8:g1 * 8],
                            Kg * 128, Kg * 128, 2 * F,
                            queue_num=qi % 4)
                        qi += 1
                    nc.vector.tensor_tensor(
                        out=msg[:].rearrange("p (q f) -> p q f", f=F),
                        in0=msg[:].rearrange("p (q f) -> p q f", f=F),
                        in1=w2b_sb[:, 2 * c0:2 * c1].to_broadcast(
                            [128, 2 * Kc, F]),
                        op=mybir.AluOpType.mult)
                    for (b, s, e, first, last) in plist:
                        o = (s - c0) * 2 * F
                        n = (e - s) * 2 * F
                        piece = msg[:, o:o + n].rearrange(
                            "p (q f) -> p f q", f=F)
                        if first:
                            agg = wp.tile([128, F], f32, tag="agg")
                            nc.vector.tensor_reduce(
                                out=agg[:], in_=piece,
                                axis=mybir.AxisListType.X,
                                op=mybir.AluOpType.add)
                        else:
                            agg = open_agg
                            tmp = wp.tile([128, F], f32, tag="aggt")
                            nc.vector.tensor_reduce(
                                out=tmp[:], in_=piece,
                                axis=mybir.AxisListType.X,
                                op=mybir.AluOpType.add)
                            nc.vector.tensor_tensor(
                                out=agg[:], in0=agg[:], in1=tmp[:],
                                op=mybir.AluOpType.add)
                        if not last:
                            open_agg = agg
                            continue
                        open_agg = None
                        # self-loop: w=1 contribution is the own h~ block
                        nc.vector.tensor_tensor(
                            out=agg[:], in0=agg[:],
                            in1=hslab[:, b * F:(b + 1) * F],
                            op=mybir.AluOpType.add)
                        nc.vector.tensor_scalar_mul(
                            agg[:], agg[:], dis_sb[:, b:b + 1])
                        tp = psA.tile([F, 128], f32, tag="tp")
                        nc.tensor.transpose(out=tp[:], in_=agg[:],
                                            identity=ident[:])
                        aug = wp.tile([F + 1, 128], f32, tag="aug")
                        nc.vector.memset(aug[F:F + 1, :], 1.0)
                        nc.vector.tensor_copy(out=aug[:F, :], in_=tp[:])
                        gps = psB.tile([128, F], f32, tag="g")
                        nc.tensor.matmul(out=gps[:], lhsT=aug[:],
                                         rhs=waug_sb[l][:],
                                         start=True, stop=True)
                        if l < 2:
                            nc.scalar.activation(
                                out=hslab[:, b * F:(b + 1) * F], in_=gps[:],
                                func=mybir.ActivationFunctionType.Relu,
                                scale=dis_sb[:, b:b + 1])
                            # ship this block to the next layer's AG bounce
                            # now, off the layer-boundary critical path
                            nc.sync.dma_start(
                                out=agins[l + 1][:].rearrange(
                                    "(bb p) f -> p bb f", p=128)[:, b:b + 1, :],
                                in_=hslab[:, b * F:(b + 1) * F])
                        else:
                            h3 = wp.tile([128, F + 1], f32, tag="h3")
                            nc.vector.memset(h3[:, F:F + 1], 1.0)
                            nc.vector.tensor_copy(out=h3[:, :F], in_=gps[:])
                            gmat = wp.tile([128, N_GRAPHS], f32, tag="gmat")
                            nc.vector.tensor_tensor(
                                out=gmat[:],
                                in0=gid_sb[:, b:b + 1].to_broadcast(
                                    [128, N_GRAPHS]),
                                in1=iota_sb[:],
                                op=mybir.AluOpType.is_equal)
                            nc.tensor.matmul(out=pool_ps[:], lhsT=gmat[:],
                                             rhs=h3[:],
                                             start=(b == 0),
                                             stop=(b == NB - 1))

            # ---- pooling epilogue ----
            poolin = wp.tile([N_GRAPHS, F + 1], f32, tag="poolin")
            nc.vector.tensor_copy(out=poolin[:], in_=pool_ps[:])
            arin = dp.tile([N_GRAPHS, F + 1], f32, tag="arin")
            arout = dp.tile([N_GRAPHS, F + 1], f32, addr_space="Shared", tag="arout")
            nc.sync.dma_start(out=arin[:], in_=poolin[:])
            nc.gpsimd.collective_compute(
                "AllReduce", mybir.AluOpType.add,
                ins=[arin[:]], outs=[arout[:]],
                replica_groups=[list(range(NC))])
            ar_sb = wp.tile([N_GRAPHS, F + 1], f32, tag="arsb")
            nc.sync.dma_start(out=ar_sb[:], in_=arout[:])
            cnt = wp.tile([N_GRAPHS, 1], f32, tag="cnt")
            nc.vector.tensor_scalar_max(cnt[:], ar_sb[:, F:F + 1], 1.0)
            rec = wp.tile([N_GRAPHS, 1], f32, tag="rec")
            nc.vector.reciprocal(out=rec[:], in_=cnt[:])
            pooled = wp.tile([N_GRAPHS, F], f32, tag="pooled")
            nc.vector.tensor_scalar_mul(pooled[:], ar_sb[:, :F], rec[:])
            tp2 = psA.tile([F, N_GRAPHS], f32, tag="tp")
            nc.tensor.transpose(out=tp2[:], in_=pooled[:],
                                identity=ident[:N_GRAPHS, :N_GRAPHS])
            aug2 = wp.tile([F + 1, N_GRAPHS], f32, tag="aug2")
            nc.vector.memset(aug2[F:F + 1, :], 1.0)
            nc.vector.tensor_copy(out=aug2[:F, :], in_=tp2[:])
            ops = psB.tile([N_GRAPHS, N_CLASSES], f32, tag="g")
            nc.tensor.matmul(out=ops[:], lhsT=aug2[:], rhs=wl_sb[:],
                             start=True, stop=True)
            out_sb = wp.tile([N_GRAPHS, N_CLASSES], f32, tag="outsb")
            nc.vector.tensor_copy(out=out_sb[:], in_=ops[:])
            nc.sync.dma_start(out=out_d[:, :], in_=out_sb[:])

    nc.compile()
    return nc


def _run(inputs, trace=False):
    x = inputs["x"]
    prep = _host_prep(x, inputs["edge_index"], inputs["batch"], inputs["P_vec"])
    key = ("nc", prep["C"], tuple(prep["Kb"]))
    if key not in _cache:
        _cache.clear()
        _cache[key] = _build(prep["Kb"], prep["cbase"], prep["C"])
    nc = _cache[key]

    waugs = []
    for (W, b) in [(inputs["W1"], inputs["b1"]), (inputs["W2"], inputs["b2"]),
                   (inputs["W3"], inputs["b3"])]:
        waugs.append(np.concatenate(
            [np.asarray(W, np.float32), np.asarray(b, np.float32)[None, :]], axis=0))
    wlaug = np.concatenate(
        [np.asarray(inputs["Wl"], np.float32),
         np.asarray(inputs["bl"], np.float32)[None, :]], axis=0)
    iota64 = np.tile(np.arange(F, dtype=np.float32)[None, :], (128, 1))

    in_maps = []
    for c in range(NC):
        in_maps.append({
            "x_own": prep["x_slots"][c],
            "idx16": prep["idx16"][c], "pv": prep["P2"][c],
            "gid": prep["gid"][c], "iota64": iota64,
            "waug0": waugs[0], "waug1": waugs[1], "waug2": waugs[2],
            "wlaug": wlaug,
        })

    res = bass_utils.run_bass_kernel_spmd(
        nc, in_maps, core_ids=list(range(NC)), trace=trace)
    return res.results[0]["out"].astype(np.float32), res


def kernel(**inputs) -> np.ndarray:
    out, _ = _run(inputs, trace=False)
    return out


# revision 38
# speedup vs baseline: 2.5907x; 1.0205x over previous
"""GCN (3-layer, edge-weighted, mean-pool, classifier) on 8 TRN2 NeuronCores.

Strategy (sharding_hint: shard nodes + incident edges across cores):
- Nodes are assigned to 8 cores round-robin by in-degree rank, so each
  core gets ~6250 nodes in 49 blocks of 128 with near-uniform in-degree
  per block.  Each target node owns K slots (its in-edges + self loop,
  padded to the block max K_b).
- norm = dis[src] * w_e * dis[tgt] is factored: dis[src] is folded into
  the gathered table (h~ = dis * h), w_e is applied per-slot on DVE,
  dis[tgt] is applied per-partition after aggregation.  The self loop
  is an ordinary slot whose weight sigmoid(30) == 1.0 exactly.
- The h~ table is bf16 and gathered in PAIRS of rows (256B elements,
  the SWDGE dma_gather granularity) with int16 indices slotrow//2; the
  unwanted half of each pair is masked by a zero weight (host pads the
  P array with -1e4, sigmoid -> 0).  One dma_gather per ~128-slot chunk
  replaces per-slot indirect DMAs (~1us fixed SWDGE cost each).
- Per layer: chunked gathers, DVE multiply+reduce, then per 128-node
  block: transpose (PE), augmented matmul with [W; b], relu*dis on ACT.
  Layers exchange h~ via AllGather.  Pooling = one-hot graph matmul
  into an accumulating PSUM bank, AllReduce, tiny classifier matmul.
"""
import sys

for p in ("/opt/trn_rl_repo", "/root/.axon_site/_ro/trn_rl_repo"):
    if p not in sys.path:
        sys.path.insert(0, p)

import numpy as np

import concourse.bacc as bacc
import concourse.bass as bass
import concourse.mybir as mybir
import concourse.tile as tile
from concourse import bass_utils
from concourse.library_config import mlp
from concourse.masks import make_identity

N_NODES = 50000
N_EDGES = 800000
F = 64
N_CLASSES = 10
N_GRAPHS = 64
NC = 8
NPC = 6272                # node slots per core (49 blocks of 128)
NB = NPC // 128           # 49
SLOTS = NC * NPC          # 50176
HALF_A = 3072             # rows per core in the first AllGather half
KCAP = 64                 # slot columns per span (DVE granularity)
GSUB = 8                  # columns per dma_gather (1024 idx ucode limit)
SELF_P = 30.0             # sigmoid(30) == 1.0 exactly in f32

_cache = {}


def _host_prep(x, edge_index, batch, P_vec):
    """Slot layout + per-core input arrays (pure index manipulation)."""
    row = np.asarray(edge_index[0], np.int64)
    col = np.asarray(edge_index[1], np.int64)
    batch = np.asarray(batch, np.int64)
    P_vec = np.asarray(P_vec, np.float32)
    x = np.asarray(x, np.float32)

    deg = np.bincount(col, minlength=N_NODES)       # self loop added on-chip
    order = np.argsort(-deg, kind="stable")         # nodes by degree desc
    r_of_node = np.empty(N_NODES, np.int64)
    r_of_node[order] = np.arange(N_NODES)
    core_of = r_of_node % NC
    pos_of = r_of_node // NC                        # < 6250
    slotrow_of = core_of * NPC + pos_of

    # graph edges only; the self loop (weight exactly 1.0) is added on-chip
    # from the core's own h~ block, so it costs no gather descriptors
    esrc = row
    etgt = col
    eP = P_vec

    # slot rank k of each edge within its target
    o = np.argsort(etgt, kind="stable")
    sk = etgt[o]
    grp_first = np.r_[True, sk[1:] != sk[:-1]]
    gstart = np.flatnonzero(grp_first)
    glen = np.diff(np.r_[gstart, len(sk)])
    kslot_sorted = np.arange(len(sk)) - np.repeat(gstart, glen)
    kslot = np.empty(len(sk), np.int64)
    kslot[o] = kslot_sorted

    # per-block chunk count (global across cores -> SPMD-uniform program)
    block_of_node = pos_of // 128
    Kb = np.zeros(NB, np.int64)
    np.maximum.at(Kb, block_of_node, deg)
    Kb = np.maximum(Kb, 1)
    cbase = np.r_[0, np.cumsum(Kb)][:-1]
    C = int(Kb.sum())

    tcore = core_of[etgt]
    tlane = pos_of[etgt] % 128
    ccol = cbase[block_of_node[etgt]] + kslot

    src_slot = slotrow_of[esrc]
    idx_arr = np.zeros((NC, 128, C), np.int32)      # pair index slotrow//2
    P2_arr = np.full((NC, 128, 2 * C), -1e4, np.float32)
    idx_arr[tcore, tlane, ccol] = src_slot // 2
    P2_arr[tcore, tlane, 2 * ccol + (src_slot % 2)] = eP

    # dma_gather int16 index layout: flat i = c*128 + p stored at
    # partition i%16, column i//16, replicated to all 8 16-row groups
    i16 = np.zeros((NC, 16, C * 8), np.int16)
    flat = np.transpose(idx_arr, (0, 2, 1)).reshape(NC, C * 128)  # (c,p)
    u16 = flat.astype(np.uint16).view(np.int16)
    ii = np.arange(C * 128)
    i16[:, ii % 16, ii // 16] = u16
    idx16 = np.tile(i16, (1, 8, 1))                  # [NC, 128, C*8]

    gid_arr = np.full((NC, 128, NB), float(N_GRAPHS), np.float32)
    gid_arr[core_of, pos_of % 128, pos_of // 128] = batch.astype(np.float32)

    x_slots = np.zeros((NC, NPC, F), np.float32)
    x_slots[core_of, pos_of] = x

    return dict(
        Kb=[int(k) for k in Kb], cbase=[int(c) for c in cbase], C=C,
        idx16=idx16, P2=P2_arr, gid=gid_arr, x_slots=x_slots,
    )


def _spans(Kb, cbase, C):
    """Column spans of <= KCAP plus per-span block pieces.

    Returns [(c0, c1, [(b, s, e, first, last), ...]), ...]: block b's
    slot columns [s, e) fall in this span; first/last flag whether the
    piece begins/ends b's range (for partial-sum stitching).
    """
    out = []
    c = 0
    while c < C:
        c0, c1 = c, min(c + KCAP, C)
        plist = []
        for b in range(NB):
            s = max(cbase[b], c0)
            e = min(cbase[b] + Kb[b], c1)
            if s < e:
                plist.append((b, s, e, s == cbase[b],
                              e == cbase[b] + Kb[b]))
        out.append((c0, c1, plist))
        c = c1
    return out


def _build(Kb, cbase, C):
    f32 = mybir.dt.float32
    bf16 = mybir.dt.bfloat16
    i16 = mybir.dt.int16
    nc = bacc.Bacc("TRN2", target_bir_lowering=False, debug=False,
                   num_devices=NC, num_swdge_queues=4)

    x_own = nc.dram_tensor("x_own", [NPC, F], f32, kind="ExternalInput")
    idx_in = nc.dram_tensor("idx16", [128, C * 8], i16, kind="ExternalInput")
    p_in = nc.dram_tensor("pv", [128, 2 * C], f32, kind="ExternalInput")
    gid_in = nc.dram_tensor("gid", [128, NB], f32, kind="ExternalInput")
    iota_in = nc.dram_tensor("iota64", [128, F], f32, kind="ExternalInput")
    waug_in = [nc.dram_tensor(f"waug{l}", [F + 1, F], f32, kind="ExternalInput")
               for l in range(3)]
    wl_in = nc.dram_tensor("wlaug", [F + 1, N_CLASSES], f32, kind="ExternalInput")
    out_d = nc.dram_tensor("out", [N_GRAPHS, N_CLASSES], f32, kind="ExternalOutput")

    spans = _spans(Kb, cbase, C)

    with tile.TileContext(nc) as tc:
        with tc.tile_pool(name="const", bufs=1) as cp, \
             tc.tile_pool(name="meta", bufs=1) as mp, \
             tc.tile_pool(name="work", bufs=3) as wp, \
             tc.tile_pool(name="msgs", bufs=4) as gp, \
             tc.tile_pool(name="psA", bufs=3, space="PSUM") as psA, \
             tc.tile_pool(name="psB", bufs=3, space="PSUM") as psB, \
             tc.tile_pool(name="psP", bufs=1, space="PSUM") as psP, \
             tc.tile_pool(name="dram", bufs=1, space="DRAM") as dp:

            nc.gpsimd.load_library(mlp)
            ident = cp.tile([128, 128], f32)
            make_identity(nc, ident[:])
            iota_sb = cp.tile([128, F], f32)
            nc.sync.dma_start(out=iota_sb[:], in_=iota_in[:, :])
            waug_sb = []
            for l in range(3):
                t = cp.tile([F + 1, F], f32, tag=f"waug{l}")
                nc.sync.dma_start(out=t[:], in_=waug_in[l][:, :])
                waug_sb.append(t)
            wl_sb = cp.tile([F + 1, N_CLASSES], f32)
            nc.sync.dma_start(out=wl_sb[:], in_=wl_in[:, :])

            idx_sb = mp.tile([128, C * 8], i16)
            nc.sync.dma_start(out=idx_sb[:], in_=idx_in[:, :])
            w2_sb = mp.tile([128, 2 * C], f32)
            w2b_sb = mp.tile([128, 2 * C], bf16)
            gid_sb = mp.tile([128, NB], f32)
            nc.sync.dma_start(out=gid_sb[:], in_=gid_in[:, :])
            dis_sb = mp.tile([128, NB], f32)
            sqd_sb = mp.tile([128, NB], f32)   # sqrt(deg)
            dis2_sb = mp.tile([128, NB], f32)  # 1/deg

            # ---- prepass: w = sigmoid(P); dis = 1/sqrt(deg_w); x~ ----
            p_sb = gp.tile([128, 2 * C], f32, tag="msg")
            nc.sync.dma_start(out=p_sb[:], in_=p_in[:, :])
            nc.scalar.activation(out=w2_sb[:], in_=p_sb[:],
                                 func=mybir.ActivationFunctionType.Sigmoid)
            nc.vector.tensor_copy(out=w2b_sb[:], in_=w2_sb[:])
            deg_sb = wp.tile([128, NB], f32, tag="deg")
            for b in range(NB):
                nc.vector.tensor_reduce(
                    out=deg_sb[:, b:b + 1],
                    in_=w2_sb[:, 2 * cbase[b]:2 * (cbase[b] + Kb[b])],
                    axis=mybir.AxisListType.X, op=mybir.AluOpType.add)
            # + 1.0 for the self loop (weight exactly 1); also keeps pad
            # lanes (deg 0) finite
            nc.scalar.activation(out=sqd_sb[:], in_=deg_sb[:],
                                 func=mybir.ActivationFunctionType.Sqrt,
                                 bias=1.0)
            nc.vector.reciprocal(out=dis_sb[:], in_=sqd_sb[:])
            nc.vector.tensor_tensor(out=dis2_sb[:], in0=dis_sb[:],
                                    in1=dis_sb[:], op=mybir.AluOpType.mult)

            # persistent own h~ blocks (bf16): self-loop source + AG payload
            hslab = mp.tile([128, NB * F], bf16)
            xs = gp.tile([128, NB * F], f32, tag="msg")
            nc.sync.dma_start(
                out=xs[:].rearrange("p (b f) -> p b f", f=F),
                in_=x_own[:, :].rearrange("(b p) f -> p b f", p=128))
            nc.vector.tensor_tensor(
                out=hslab[:].rearrange("p (b f) -> p b f", f=F),
                in0=xs[:].rearrange("p (b f) -> p b f", f=F),
                in1=dis_sb[:].to_broadcast([128, NB, F]),
                op=mybir.AluOpType.mult)

            agins = [dp.tile([NPC, F], bf16, name=f"agin{l}", tag=f"agin{l}")
                     for l in range(3)]
            agouts = [dp.tile([SLOTS, F], bf16, addr_space="Shared",
                              name=f"agout{l}", tag=f"ag{l}") for l in range(3)]
            pool_ps = psP.tile([N_GRAPHS, F + 1], f32)

            nc.sync.dma_start(
                out=agins[0][:].rearrange("(b p) f -> p b f", p=128),
                in_=hslab[:].rearrange("p (b f) -> p b f", f=F))
            qi = 0
            for l in range(3):
                # layer 1/2 bounce buffers were already filled per-block by
                # the previous layer's epilogue DMAs
                nc.gpsimd.collective_compute(
                    "AllGather", mybir.AluOpType.bypass,
                    ins=[agins[l][:]], outs=[agouts[l][:]],
                    replica_groups=[list(range(NC))])
                # gather source viewed as pair rows [SLOTS/2, 2F]
                src = agouts[l][:].rearrange("(j t) f -> j (t f)", t=2)

                open_agg = None
                for (c0, c1, plist) in spans:
                    Kc = c1 - c0
                    msg = gp.tile([128, Kc * 2 * F], bf16, tag="msg")
                    for g0 in range(c0, c1, GSUB):
                        g1 = min(g0 + GSUB, c1)
                        Kg = g1 - g0
                        nc.gpsimd.dma_gather(
                            msg[:, (g0 - c0) * 2 * F:(g1 - c0) * 2 * F]
                            .rearrange("p (k f) -> p k f", f=2 * F),
                            src,
                            idx_sb[:, g0 * 8:g1 * 8],
                            Kg * 128, Kg * 128, 2 * F,
                            queue_num=qi % 4)
                        qi += 1
                    nc.vector.tensor_tensor(
                        out=msg[:].rearrange("p (q f) -> p q f", f=F),
                        in0=msg[:].rearrange("p (q f) -> p q f", f=F),
                        in1=w2b_sb[:, 2 * c0:2 * c1].to_broadcast(
                            [128, 2 * Kc, F]),
                        op=mybir.AluOpType.mult)
                    for (b, s, e, first, last) in plist:
                        o = (s - c0) * 2 * F
                        n = (e - s) * 2 * F
                        piece = msg[:, o:o + n].rearrange(
                            "p (q f) -> p f q", f=F)
                        if first:
                            agg = wp.tile([128, F], f32, tag="agg")
                            nc.vector.tensor_reduce(
                                out=agg[:], in_=piece,
                                axis=mybir.AxisListType.X,
                                op=mybir.AluOpType.add)
                        else:
                            agg = open_agg
                            tmp = wp.tile([128, F], f32, tag="aggt")
                            nc.vector.tensor_reduce(
                                out=tmp[:], in_=piece,
                                axis=mybir.AxisListType.X,
                                op=mybir.AluOpType.add)
                            nc.vector.tensor_tensor(
                                out=agg[:], in0=agg[:], in1=tmp[:],
                                op=mybir.AluOpType.add)
                        if not last:
                            open_agg = agg
                            continue
                        open_agg = None
                        # self-loop: w=1 contribution is the own h~ block
                        nc.vector.tensor_tensor(
                            out=agg[:], in0=agg[:],
                            in1=hslab[:, b * F:(b + 1) * F],
                            op=mybir.AluOpType.add)
                        nc.vector.tensor_scalar_mul(
                            agg[:], agg[:], dis_sb[:, b:b + 1])
                        tp = psA.tile([F, 128], f32, tag="tp")
                        nc.tensor.transpose(out=tp[:], in_=agg[:],
                                            identity=ident[:])
                        aug = wp.tile([F + 1, 128], f32, tag="aug")
                        nc.vector.memset(aug[F:F + 1, :], 1.0)
                        nc.vector.tensor_copy(out=aug[:F, :], in_=tp[:])
                        gps = psB.tile([128, F], f32, tag="g")
                        nc.tensor.matmul(out=gps[:], lhsT=aug[:],
                                         rhs=waug_sb[l][:],
                                         start=True, stop=True)
                        if l < 2:
                            nc.scalar.activation(
                                out=hslab[:, b * F:(b + 1) * F], in_=gps[:],
                                func=mybir.ActivationFunctionType.Relu,
                                scale=dis_sb[:, b:b + 1])
                            # ship this block to the next layer's AG bounce
                            # now, off the layer-boundary critical path
                            nc.sync.dma_start(
                                out=agins[l + 1][:].rearrange(
                                    "(bb p) f -> p bb f", p=128)[:, b:b + 1, :],
                                in_=hslab[:, b * F:(b + 1) * F])
                        else:
                            h3 = wp.tile([128, F + 1], f32, tag="h3")
                            nc.vector.memset(h3[:, F:F + 1], 1.0)
                            nc.vector.tensor_copy(out=h3[:, :F], in_=gps[:])
                            gmat = wp.tile([128, N_GRAPHS], f32, tag="gmat")
                            nc.vector.tensor_tensor(
                                out=gmat[:],
                                in0=gid_sb[:, b:b + 1].to_broadcast(
                                    [128, N_GRAPHS]),
                                in1=iota_sb[:],
                                op=mybir.AluOpType.is_equal)
                            nc.tensor.matmul(out=pool_ps[:], lhsT=gmat[:],
                                             rhs=h3[:],
                                             start=(b == 0),
                                             stop=(b == NB - 1))

            # ---- pooling epilogue ----
            poolin = wp.tile([N_GRAPHS, F + 1], f32, tag="poolin")
            nc.vector.tensor_copy(out=poolin[:], in_=pool_ps[:])
            arin = dp.tile([N_GRAPHS, F + 1], f32, tag="arin")
            arout = dp.tile([N_GRAPHS, F + 1], f32, addr_space="Shared", tag="arout")
            nc.sync.dma_start(out=arin[:], in_=poolin[:])
            nc.gpsimd.collective_compute(
                "AllReduce", mybir.AluOpType.add,
                ins=[arin[:]], outs=[arout[:]],
                replica_groups=[list(range(NC))])
            ar_sb = wp.tile([N_GRAPHS, F + 1], f32, tag="arsb")
            nc.sync.dma_start(out=ar_sb[:], in_=arout[:])
            cnt = wp.tile([N_GRAPHS, 1], f32, tag="cnt")
            nc.vector.tensor_scalar_max(cnt[:], ar_sb[:, F:F + 1], 1.0)
            rec = wp.tile([N_GRAPHS, 1], f32, tag="rec")
            nc.vector.reciprocal(out=rec[:], in_=cnt[:])
            pooled = wp.tile([N_GRAPHS, F], f32, tag="pooled")
            nc.vector.tensor_scalar_mul(pooled[:], ar_sb[:, :F], rec[:])
            tp2 = psA.tile([F, N_GRAPHS], f32, tag="tp")
            nc.tensor.transpose(out=tp2[:], in_=pooled[:],
                                identity=ident[:N_GRAPHS, :N_GRAPHS])
            aug2 = wp.tile([F + 1, N_GRAPHS], f32, tag="aug2")
            nc.vector.memset(aug2[F:F + 1, :], 1.0)
            nc.vector.tensor_copy(out=aug2[:F, :], in_=tp2[:])
            ops = psB.tile([N_GRAPHS, N_CLASSES], f32, tag="g")
            nc.tensor.matmul(out=ops[:], lhsT=aug2[:], rhs=wl_sb[:],
                             start=True, stop=True)
            out_sb = wp.tile([N_GRAPHS, N_CLASSES], f32, tag="outsb")
            nc.vector.tensor_copy(out=out_sb[:], in_=ops[:])
            nc.sync.dma_start(out=out_d[:, :], in_=out_sb[:])

    nc.compile()
    return nc


def _run(inputs, trace=False):
    x = inputs["x"]
    prep = _host_prep(x, inputs["edge_index"], inputs["batch"], inputs["P_vec"])
    key = ("nc", prep["C"], tuple(prep["Kb"]))
    if key not in _cache:
        _cache.clear()
        _cache[key] = _build(prep["Kb"], prep["cbase"], prep["C"])
    nc = _cache[key]

    waugs = []
    for (W, b) in [(inputs["W1"], inputs["b1"]), (inputs["W2"], inputs["b2"]),
                   (inputs["W3"], inputs["b3"])]:
        waugs.append(np.concatenate(
            [np.asarray(W, np.float32), np.asarray(b, np.float32)[None, :]], axis=0))
    wlaug = np.concatenate(
        [np.asarray(inputs["Wl"], np.float32),
         np.asarray(inputs["bl"], np.float32)[None, :]], axis=0)
    iota64 = np.tile(np.arange(F, dtype=np.float32)[None, :], (128, 1))

    in_maps = []
    for c in range(NC):
        in_maps.append({
            "x_own": prep["x_slots"][c],
            "idx16": prep["idx16"][c], "pv": prep["P2"][c],
            "gid": prep["gid"][c], "iota64": iota64,
            "waug0": waugs[0], "waug1": waugs[1], "waug2": waugs[2],
            "wlaug": wlaug,
        })

    res = bass_utils.run_bass_kernel_spmd(
        nc, in_maps, core_ids=list(range(NC)), trace=trace)
    return res.results[0]["out"].astype(np.float32), res


def kernel(**inputs) -> np.ndarray:
    out, _ = _run(inputs, trace=False)
    return out
